# revision 1
# baseline (speedup 1.0000x reference)
"""GCN layer on 8 Trainium2 NeuronCores.

out = relu(D^{-1/2} (A+I) D^{-1/2} x W^T + b),  N=8192, D=512, A symmetric binary.

Sharding (1-D graph partition, rank c owns nodes [c*1024, (c+1)*1024)):
  - Because A+I is symmetric, the row-block (A+I)[own, :] the core must
    aggregate equals the column slab (A+I)[:, own] transposed — so each core is
    fed the NATURAL column slab, which is exactly the [K, M]/[K, N] layout the
    PE array wants. No transposes anywhere.
  - Device computes: deg partials (DVE rowsums of the slab) -> AllReduce (full
    deg, for scaling neighbor features) + ReduceScatter (own deg, keeps the
    program SPMD-uniform) -> y = d^{-1/2} x (bf16) -> hT = yT @ slab (PE,
    y stationary) -> out = relu(d_own^{-1/2} * (hT^T @ W^T) + b) (PE + fused
    per-partition scale/relu on evac).
  - A is binary so the bf16 slab is exact; x/W are rounded to bf16 (the PE's
    fp32 path is 4x slower and this problem is memory-target).
"""

import numpy as np

N = 8192
D = 512
NCORES = 8
B = N // NCORES          # 1024 nodes per core
P = 128
KT = N // P              # 64 k-tiles of 128 rows
NCH = 8                  # slab chunks (8 k-tiles each)
KPC = KT // NCH          # k-tiles per chunk

_cache = {}


def _build(with_bias: bool, ar_chunks: int = 1):
    import concourse.tile as tile
    from concourse import bacc, mybir

    f32 = mybir.dt.float32
    bf16 = mybir.dt.bfloat16

    nc = bacc.Bacc("TRN2", target_bir_lowering=False, debug=False,
                   num_devices=NCORES)

    slab_d = nc.dram_tensor("slab", [N, B], f32, kind="ExternalInput").ap()
    x_d = nc.dram_tensor("x", [N, D], f32, kind="ExternalInput").ap()
    wt_d = nc.dram_tensor("wt", [D, D], f32, kind="ExternalInput").ap()
    if with_bias:
        bb_d = nc.dram_tensor("bb", [P, D], f32, kind="ExternalInput").ap()
    out_d = nc.dram_tensor("out", [B, D], f32, kind="ExternalOutput").ap()

    assert KT % ar_chunks == 0
    kt_per_ar = KT // ar_chunks

    with tile.TileContext(nc) as tc:
        with tc.tile_pool(name="slab", bufs=1) as slab_pool, \
             tc.tile_pool(name="y", bufs=3) as y_pool, \
             tc.tile_pool(name="small", bufs=1) as small, \
             tc.tile_pool(name="osb", bufs=3) as osb_pool, \
             tc.tile_pool(name="psum", bufs=1, space="PSUM") as psum_pool, \
             tc.tile_pool(name="dram", bufs=1, space="DRAM") as dram:

            # ---- resident bf16 slab, DMA-cast from fp32 HBM ----
            slab_sb = []
            for ch in range(NCH):
                t = slab_pool.tile([P, KPC, B], bf16, name=f"slab{ch}")
                src = slab_d[ch * (KPC * P):(ch + 1) * (KPC * P), :]
                nc.gpsimd.dma_start(t[:], src.rearrange("(n p) f -> p n f", p=P))
                slab_sb.append(t)

            # ---- deg partials: rowsums of the slab (per 128-row k-tile) ----
            partials = small.tile([P, KT], f32, name="partials")
            for k in range(KT):
                ch, i = divmod(k, KPC)
                nc.vector.reduce_sum(partials[:, k:k + 1], slab_sb[ch][:, i, :],
                                     axis=mybir.AxisListType.X)

            # ---- collectives: full deg (AllReduce) + own deg (ReduceScatter)
            # node-ordered DRAM bounce: flat index = k*128 + p = node id
            bounce = dram.tile([N], f32, name="bounce")
            deg_all_d = dram.tile([N], f32, name="deg_all")
            deg_own_d = dram.tile([B], f32, name="deg_own")
            rg = [list(range(NCORES))]

            nc.sync.dma_start(bounce[:].rearrange("(k p) -> p k", p=P), partials[:])
            nc.gpsimd.collective_compute(
                "ReduceScatter", mybir.AluOpType.add, replica_groups=rg,
                ins=[bounce.opt()], outs=[deg_own_d.opt()])
            for g in range(ar_chunks):
                sl = slice(g * kt_per_ar * P, (g + 1) * kt_per_ar * P)
                nc.gpsimd.collective_compute(
                    "AllReduce", mybir.AluOpType.add, replica_groups=rg,
                    ins=[bounce[sl].opt()], outs=[deg_all_d[sl].opt()])

            # ---- d^{-1/2} = sqrt(1/deg)  (deg >= 1 always) ----
            deg_all = small.tile([P, KT], f32, name="deg_all_sb")
            dinv_all = small.tile([P, KT], f32, name="dinv_all")
            for g in range(ar_chunks):
                ks = slice(g * kt_per_ar, (g + 1) * kt_per_ar)
                fl = slice(g * kt_per_ar * P, (g + 1) * kt_per_ar * P)
                nc.sync.dma_start(deg_all[:, ks],
                                  deg_all_d[fl].rearrange("(k p) -> p k", p=P))
                nc.vector.reciprocal(dinv_all[:, ks], deg_all[:, ks])
                nc.scalar.sqrt(dinv_all[:, ks], dinv_all[:, ks])

            deg_own = small.tile([P, NCH], f32, name="deg_own_sb")
            dinv_own = small.tile([P, NCH], f32, name="dinv_own")
            nc.sync.dma_start(deg_own[:], deg_own_d[:].rearrange("(m p) -> p m", p=P))
            nc.vector.reciprocal(dinv_own[:], deg_own[:])
            nc.scalar.sqrt(dinv_own[:], dinv_own[:])

            if with_bias:
                bb = small.tile([P, D], f32, name="bb_sb")
                nc.sync.dma_start(bb[:], bb_d[:])

            # ---- W^T, bf16 ----
            wt_sb = small.tile([P, D // P, D], bf16, name="wt_sb")
            nc.gpsimd.dma_start(wt_sb[:], wt_d.rearrange("(kf p) f -> p kf f", p=P))

            # ---- main matmul: hT[feat, own] = yT @ slabI, accumulated over k
            hT_ps = [psum_pool.tile([P, 512], mybir.dt.float32, name=f"ps_{j}",
                                    tag=f"ps_{j}") for j in range(8)]
            for ch in range(NCH):
                y_t = y_pool.tile([P, KPC, D], bf16, tag="y")
                src = x_d[ch * (KPC * P):(ch + 1) * (KPC * P), :]
                nc.gpsimd.dma_start(y_t[:], src.rearrange("(n p) f -> p n f", p=P))
                for i in range(KPC):
                    k = ch * KPC + i
                    nc.vector.tensor_scalar_mul(y_t[:, i, :], y_t[:, i, :],
                                                dinv_all[:, k:k + 1])
                    for mf in range(4):
                        lhs = y_t[:, i, mf * P:(mf + 1) * P]
                        for h in range(2):
                            nc.tensor.matmul(
                                hT_ps[mf * 2 + h],
                                lhsT=lhs,
                                rhs=slab_sb[ch][:, i, h * 512:(h + 1) * 512],
                                start=(k == 0), stop=(k == KT - 1))

            # evacuate hT -> bf16 SBUF [feat_part, 4, own(1024)]
            hT_sb = small.tile([P, 4, B], bf16, name="hT_sb")
            for mf in range(4):
                for h in range(2):
                    nc.vector.tensor_copy(hT_sb[:, mf, h * 512:(h + 1) * 512],
                                          hT_ps[mf * 2 + h][:])

            # ---- out = relu(d_own^{-1/2} * (hT^T @ W^T) + b) ----
            out_r = out_d.rearrange("(m p) f -> p m f", p=P)
            for m in range(NCH):
                o_ps = psum_pool.tile([P, D], mybir.dt.float32, name=f"ops_{m}",
                                      tag=f"ps_{m}")
                for kf in range(4):
                    nc.tensor.matmul(o_ps, lhsT=hT_sb[:, kf, m * P:(m + 1) * P],
                                     rhs=wt_sb[:, kf, :],
                                     start=(kf == 0), stop=(kf == 3))
                o_sb = osb_pool.tile([P, D], f32, tag="osb")
                if with_bias:
                    nc.vector.tensor_scalar_mul(o_sb[:], o_ps[:],
                                                dinv_own[:, m:m + 1])
                    nc.vector.tensor_add(o_sb[:], o_sb[:], bb[:])
                    nc.vector.tensor_scalar_max(o_sb[:], o_sb[:], 0.0)
                else:
                    nc.vector.tensor_scalar(o_sb[:], o_ps[:],
                                            dinv_own[:, m:m + 1], 0.0,
                                            mybir.AluOpType.mult,
                                            mybir.AluOpType.max)
                nc.sync.dma_start(out_r[:, m, :], o_sb[:])

    nc.compile()
    return nc


def _prep_in_maps(x, A, W, b, with_bias):
    eye_add = np.arange(N)
    xs = np.ascontiguousarray(x, dtype=np.float32)
    wt = np.ascontiguousarray(W.T, dtype=np.float32)
    in_maps = []
    for c in range(NCORES):
        sl = np.array(A[:, c * B:(c + 1) * B], dtype=np.float32)
        # fold the +I of A_tilde = A + I into the fed slab (host-side graph prep)
        idx = eye_add[c * B:(c + 1) * B]
        sl[idx, np.arange(B)] += 1.0
        m = {"slab": sl, "x": xs, "wt": wt}
        if with_bias:
            m["bb"] = np.ascontiguousarray(
                np.broadcast_to(b.astype(np.float32), (P, D)))
        in_maps.append(m)
    return in_maps


def get_compiled(with_bias, ar_chunks=1):
    key = (with_bias, ar_chunks)
    if key not in _cache:
        _cache[key] = _build(with_bias, ar_chunks)
    return _cache[key]


def kernel(x, A, W, b):
    from concourse import bass_utils

    with_bias = bool(np.any(b))
    nc = get_compiled(with_bias)
    in_maps = _prep_in_maps(x, A, W, b, with_bias)
    res = bass_utils.run_bass_kernel_spmd(nc, in_maps, core_ids=list(range(NCORES)))
    out = np.concatenate([res.results[c]["out"] for c in range(NCORES)], axis=0)
    return out.astype(np.float32)


# revision 3
# speedup vs baseline: 21.8839x; 21.8839x over previous
"""GCN layer on 8 Trainium2 NeuronCores.

out = relu(D^{-1/2} (A+I) D^{-1/2} x W^T + b),  N=8192, D=512, A symmetric binary.

Sharding (1-D graph partition, rank c owns nodes [c*1024, (c+1)*1024)):
  - Because A+I is symmetric, the row-block (A+I)[own, :] the core must
    aggregate equals the column slab (A+I)[:, own] transposed — so each core is
    fed the NATURAL column slab, which is exactly the [K, M]/[K, N] layout the
    PE array wants. No transposes anywhere.
  - Device computes: deg partials (DVE rowsums of the slab) -> AllReduce (full
    deg, for scaling neighbor features) + ReduceScatter (own deg, keeps the
    program SPMD-uniform) -> y = d^{-1/2} x (bf16) -> hT = yT @ slab (PE,
    y stationary) -> out = relu(d_own^{-1/2} * (hT^T @ W^T) + b) (PE + fused
    per-partition scale/relu on evac).
  - A is binary so the bf16 slab is exact; x/W are rounded to bf16 (the PE's
    fp32 path is 4x slower and this problem is memory-target).
"""

import numpy as np

N = 8192
D = 512
NCORES = 8
B = N // NCORES          # 1024 nodes per core
P = 128
KT = N // P              # 64 k-tiles of 128 rows
NCH = 8                  # slab chunks (8 k-tiles each)
KPC = KT // NCH          # k-tiles per chunk

_cache = {}


def _build(with_bias: bool, ar_chunks: int = 1, reps: int = 1):
    import concourse.tile as tile
    from concourse import bacc, mybir

    f32 = mybir.dt.float32
    bf16 = mybir.dt.bfloat16

    nc = bacc.Bacc("TRN2", target_bir_lowering=False, debug=False,
                   num_devices=NCORES)

    slab_d = nc.dram_tensor("slab", [N, B], f32, kind="ExternalInput").ap()
    x_d = nc.dram_tensor("x", [N, D], f32, kind="ExternalInput").ap()
    wt_d = nc.dram_tensor("wt", [D, D], f32, kind="ExternalInput").ap()
    if with_bias:
        bb_d = nc.dram_tensor("bb", [P, D], f32, kind="ExternalInput").ap()
    out_d = nc.dram_tensor("out", [B, D], f32, kind="ExternalOutput").ap()

    assert KT % ar_chunks == 0
    kt_per_ar = KT // ar_chunks

    with tile.TileContext(nc) as tc:
        with tc.tile_pool(name="slab", bufs=1) as slab_pool, \
             tc.tile_pool(name="y", bufs=3) as y_pool, \
             tc.tile_pool(name="small", bufs=1) as small, \
             tc.tile_pool(name="osb", bufs=3) as osb_pool, \
             tc.tile_pool(name="psum", bufs=1, space="PSUM") as psum_pool, \
             tc.tile_pool(name="dram", bufs=1, space="DRAM") as dram:
          for _rep in range(reps):
            # ---- resident bf16 slab, DMA-cast from fp32 HBM ----
            slab_sb = []
            for ch in range(NCH):
                t = slab_pool.tile([P, KPC, B], bf16, name=f"slab{ch}")
                src = slab_d[ch * (KPC * P):(ch + 1) * (KPC * P), :]
                nc.gpsimd.dma_start(t[:], src.rearrange("(n p) f -> p n f", p=P))
                slab_sb.append(t)

            # ---- deg partials: rowsums of the slab (per 128-row k-tile) ----
            partials = small.tile([P, KT], f32, name="partials")
            for k in range(KT):
                ch, i = divmod(k, KPC)
                nc.vector.reduce_sum(partials[:, k:k + 1], slab_sb[ch][:, i, :],
                                     axis=mybir.AxisListType.X)

            # ---- collectives: full deg (AllReduce) + own deg (ReduceScatter)
            # node-ordered DRAM bounce: flat index = k*128 + p = node id
            bounce = dram.tile([N], f32, name="bounce")
            deg_all_d = dram.tile([N], f32, name="deg_all")
            deg_own_d = dram.tile([B], f32, name="deg_own")
            rg = [list(range(NCORES))]

            nc.sync.dma_start(bounce[:].rearrange("(k p) -> p k", p=P), partials[:])
            nc.gpsimd.collective_compute(
                "ReduceScatter", mybir.AluOpType.add, replica_groups=rg,
                ins=[bounce.opt()], outs=[deg_own_d.opt()])
            for g in range(ar_chunks):
                sl = slice(g * kt_per_ar * P, (g + 1) * kt_per_ar * P)
                nc.gpsimd.collective_compute(
                    "AllReduce", mybir.AluOpType.add, replica_groups=rg,
                    ins=[bounce[sl].opt()], outs=[deg_all_d[sl].opt()])

            # ---- d^{-1/2} = sqrt(1/deg)  (deg >= 1 always) ----
            deg_all = small.tile([P, KT], f32, name="deg_all_sb")
            dinv_all = small.tile([P, KT], f32, name="dinv_all")
            for g in range(ar_chunks):
                ks = slice(g * kt_per_ar, (g + 1) * kt_per_ar)
                fl = slice(g * kt_per_ar * P, (g + 1) * kt_per_ar * P)
                nc.sync.dma_start(deg_all[:, ks],
                                  deg_all_d[fl].rearrange("(k p) -> p k", p=P))
                nc.vector.reciprocal(dinv_all[:, ks], deg_all[:, ks])
                nc.scalar.sqrt(dinv_all[:, ks], dinv_all[:, ks])

            deg_own = small.tile([P, NCH], f32, name="deg_own_sb")
            dinv_own = small.tile([P, NCH], f32, name="dinv_own")
            nc.sync.dma_start(deg_own[:], deg_own_d[:].rearrange("(m p) -> p m", p=P))
            nc.vector.reciprocal(dinv_own[:], deg_own[:])
            nc.scalar.sqrt(dinv_own[:], dinv_own[:])

            if with_bias:
                bb = small.tile([P, D], f32, name="bb_sb")
                nc.sync.dma_start(bb[:], bb_d[:])

            # ---- W^T, bf16 ----
            wt_sb = small.tile([P, D // P, D], bf16, name="wt_sb")
            nc.gpsimd.dma_start(wt_sb[:], wt_d.rearrange("(kf p) f -> p kf f", p=P))

            # ---- main matmul: hT[feat, own] = yT @ slabI, accumulated over k
            hT_ps = [psum_pool.tile([P, 512], mybir.dt.float32, name=f"ps_{j}",
                                    tag=f"ps_{j}") for j in range(8)]
            for ch in range(NCH):
                y_t = y_pool.tile([P, KPC, D], bf16, tag="y")
                src = x_d[ch * (KPC * P):(ch + 1) * (KPC * P), :]
                nc.gpsimd.dma_start(y_t[:], src.rearrange("(n p) f -> p n f", p=P))
                for i in range(KPC):
                    k = ch * KPC + i
                    nc.vector.tensor_scalar_mul(y_t[:, i, :], y_t[:, i, :],
                                                dinv_all[:, k:k + 1])
                    for mf in range(4):
                        lhs = y_t[:, i, mf * P:(mf + 1) * P]
                        for h in range(2):
                            nc.tensor.matmul(
                                hT_ps[mf * 2 + h],
                                lhsT=lhs,
                                rhs=slab_sb[ch][:, i, h * 512:(h + 1) * 512],
                                start=(k == 0), stop=(k == KT - 1))

            # evacuate hT -> bf16 SBUF [feat_part, 4, own(1024)]
            hT_sb = small.tile([P, 4, B], bf16, name="hT_sb")
            for mf in range(4):
                for h in range(2):
                    nc.vector.tensor_copy(hT_sb[:, mf, h * 512:(h + 1) * 512],
                                          hT_ps[mf * 2 + h][:])

            # ---- out = relu(d_own^{-1/2} * (hT^T @ W^T) + b) ----
            out_r = out_d.rearrange("(m p) f -> p m f", p=P)
            for m in range(NCH):
                o_ps = psum_pool.tile([P, D], mybir.dt.float32, name=f"ops_{m}",
                                      tag=f"ps_{m}")
                for kf in range(4):
                    nc.tensor.matmul(o_ps, lhsT=hT_sb[:, kf, m * P:(m + 1) * P],
                                     rhs=wt_sb[:, kf, :],
                                     start=(kf == 0), stop=(kf == 3))
                o_sb = osb_pool.tile([P, D], f32, tag="osb")
                if with_bias:
                    nc.vector.tensor_scalar_mul(o_sb[:], o_ps[:],
                                                dinv_own[:, m:m + 1])
                    nc.vector.tensor_add(o_sb[:], o_sb[:], bb[:])
                    nc.vector.tensor_scalar_max(o_sb[:], o_sb[:], 0.0)
                else:
                    nc.vector.tensor_scalar(o_sb[:], o_ps[:],
                                            dinv_own[:, m:m + 1], 0.0,
                                            mybir.AluOpType.mult,
                                            mybir.AluOpType.max)
                nc.sync.dma_start(out_r[:, m, :], o_sb[:])

    nc.compile()
    return nc


def _prep_in_maps(x, A, W, b, with_bias):
    eye_add = np.arange(N)
    xs = np.ascontiguousarray(x, dtype=np.float32)
    wt = np.ascontiguousarray(W.T, dtype=np.float32)
    in_maps = []
    for c in range(NCORES):
        sl = np.array(A[:, c * B:(c + 1) * B], dtype=np.float32)
        # fold the +I of A_tilde = A + I into the fed slab (host-side graph prep)
        idx = eye_add[c * B:(c + 1) * B]
        sl[idx, np.arange(B)] += 1.0
        m = {"slab": sl, "x": xs, "wt": wt}
        if with_bias:
            m["bb"] = np.ascontiguousarray(
                np.broadcast_to(b.astype(np.float32), (P, D)))
        in_maps.append(m)
    return in_maps


def get_compiled(with_bias, ar_chunks=1, reps=1):
    key = (with_bias, ar_chunks, reps)
    if key not in _cache:
        _cache[key] = _build(with_bias, ar_chunks, reps)
    return _cache[key]


def kernel(x, A, W, b):
    from concourse import bass_utils

    with_bias = bool(np.any(b))
    nc = get_compiled(with_bias)
    in_maps = _prep_in_maps(x, A, W, b, with_bias)
    res = bass_utils.run_bass_kernel_spmd(nc, in_maps, core_ids=list(range(NCORES)))
    out = np.concatenate([res.results[c]["out"] for c in range(NCORES)], axis=0)
    return out.astype(np.float32)
